# revision 23
# baseline (speedup 1.0000x reference)
"""Multi-head self-attention block on Trainium2, 8-core SPMD.

Problem (fixed shapes): x(2,2048,1024), causal-additive mask(2,2048,2048),
w_qkv(1024,3072), b_qkv(3072), w_out(1024,1024), b_out(1024).
out = MHSA(x) with H=16 heads, head_dim=64.

Fast causal path:
  - Tensor-parallel over heads (2 heads/core) for QKV + attention;
    token-parallel out projection after AllToAll exchanges.
  - All matmul operands are bf16 (x, w_qkv, q, k, v, exp-scores, att,
    w_out); PSUM accumulation and the softmax stay fp32.
  - One software-pipelined instruction stream: QKV-projection and
    out-projection matmuls are injected as "fillers" between attention
    chunks so the PE never idles (TRN2's HAM throttle re-clocks the PE
    down after any idle window; gaps are poison).
  - Scores per 128-key chunk land in a manually-rotated 3-bank PSUM
    tile; ONE activation computes exp for both heads via a strided AP
    (~80 instead of 160 ACT instructions). The two per-head score
    matmuls use contraction rows 0-63/64-127 so the PE runs them
    concurrently as row-tiles.
  - Causal: diagonal chunks restrict the moving dim to the triangle
    (multiples of 128), and masking is a multiplicative bf16 0/1
    DVE multiply on the exp outputs.
  - Softmax denominator comes from an all-ones column in V (fused into
    the AV matmul). 1/d: the denominator row is DMA-scattered to a
    [128,8] column layout so the (8-cycle-per-row iterative) DVE
    reciprocal uses all partitions, DMA-gathered back, then broadcast
    across partitions on the idle GpSimd engine; one DVE multiply per
    head produces the bf16 att tile.
  - Blocks run batch-interleaved (0,0),(1,0),(0,1),(1,1),(0,2),(0,3),
    (1,2),(1,3) and the AllToAll is split in three (blocks 0-3 / 4-5 /
    6-7, bf16 payloads) so the first two exchanges fully overlap
    compute; deferred out-projection chunks bridge the tail collective.
  - The out-projection bias is added during the PSUM->SBUF move against
    a partition-broadcast bias tile (no rank-1 bias matmuls).
Fallback: the original general/dense builder handles non-causal masks.
"""

import os
import sys
from collections import deque
from contextlib import ExitStack

if "/opt/trn_rl_repo" not in sys.path:
    sys.path.insert(0, "/opt/trn_rl_repo")

import numpy as np

import concourse.mybir as mybir
import concourse.tile as tile
from concourse import bacc, bass_utils

B, S, D, H, HD = 2, 2048, 1024, 16, 64
NCORES = 8
SL = B * S            # 4096 tokens total
QC = 512              # q-chunk
KC = 128              # k-chunk (partition dim)
NQ = S // QC          # 4 q-chunks per batch
DK = D // 128         # 8 contraction chunks of the model dim
VW = 2 * (HD + 1)     # 130: V-natural block width (2 heads x (64 V + ones))

f32 = mybir.dt.float32
f32r = mybir.dt.float32r
bf16 = mybir.dt.bfloat16
FX = mybir.ActivationFunctionType
ALU = mybir.AluOpType

# completion order of attention blocks (batch-interleaved) and the qkv
# token-chunk each one unlocks (t = 4*b + j)
BLOCK_ORDER = [(0, 0), (1, 0), (0, 1), (1, 1), (0, 2), (0, 3), (1, 2), (1, 3)]
T_ORDER = [4 * b + j for (b, j) in BLOCK_ORDER]
# the AllToAll is split three ways so the exchanges overlap compute and
# only a small tail collective remains: blocks 0-3, then 4-5, then 6-7.
CC_GROUPS = [(0, 4), (4, 6), (6, 8)]          # [bi_start, bi_end)
CC_TOK = [512 * (e - s) // NCORES for (s, e) in CC_GROUPS]   # 256,128,128
CC_ROW0 = [0, 256, 384]                        # output row offsets

LAST_EXEC_NS = None   # HW exec time (ns) of the last traced run
LAST_RESULTS = None


def _build_causal():
    nc = bacc.Bacc("TRN2", target_bir_lowering=False, debug=False,
                   num_devices=NCORES)

    xT_d = nc.dram_tensor("xT", [D, SL], bf16, kind="ExternalInput")
    wqkv_d = nc.dram_tensor("wqkv", [D, 384], bf16, kind="ExternalInput")
    bqkv_d = nc.dram_tensor("bqkv", [128, 3], f32, kind="ExternalInput")
    wout_d = nc.dram_tensor("wout", [D, D], bf16, kind="ExternalInput")
    bout_d = nc.dram_tensor("bout", [1, D], f32, kind="ExternalInput")
    ident_d = nc.dram_tensor("ident", [128, 128], bf16, kind="ExternalInput")
    masks_d = nc.dram_tensor("masks", [128, 384], bf16, kind="ExternalInput")
    out_d = nc.dram_tensor("out", [QC, D], f32, kind="ExternalOutput")

    with tile.TileContext(nc) as tc:
        with ExitStack() as stack:
            ep = stack.enter_context
            cpool = ep(tc.tile_pool(name="consts", bufs=1))
            big = ep(tc.tile_pool(name="big", bufs=1))
            xpool = ep(tc.tile_pool(name="xts", bufs=16))
            vpool = ep(tc.tile_pool(name="vstg", bufs=2))
            rpool = ep(tc.tile_pool(name="rpool", bufs=3))
            bspool = ep(tc.tile_pool(name="bspool", bufs=2))
            apool = ep(tc.tile_pool(name="apool", bufs=2))
            ppool = ep(tc.tile_pool(name="ppool", bufs=2))
            opool = ep(tc.tile_pool(name="opool", bufs=2))
            dram = ep(tc.tile_pool(name="dram", bufs=1, space="DRAM"))
            ps_s = ep(tc.tile_pool(name="ps_s", bufs=1, space="PSUM"))
            ps_q = ep(tc.tile_pool(name="ps_q", bufs=1, space="PSUM"))
            psav = ep(tc.tile_pool(name="psav", bufs=2, space="PSUM"))

            # ---------------- constants / resident tensors ----------------
            w_sb = big.tile([128, DK * 384], bf16, name="w_sb")
            qT = big.tile([128, SL], bf16, name="qT")
            kT = big.tile([128, SL], bf16, name="kT")
            vn = big.tile([128, B * (S // KC) * VW], bf16, name="vn")
            wo_sb = big.tile([128, DK * D], bf16, name="wo_sb")
            e_all = big.tile([128, 6 * QC], bf16, name="e_all")
            s_all = ps_s.tile([128, 3 * QC], f32, name="s_all")
            ident = cpool.tile([128, 128], bf16, name="ident")
            bq_sb = cpool.tile([128, 3], f32, name="bq_sb")
            masks_sb = cpool.tile([128, 384], bf16, name="masks_sb")
            bo_sb = cpool.tile([1, D], f32, name="bo_sb")
            bo_bc = big.tile([128, D], f32, name="bo_bc")

            vn_ones = vn[:].rearrange("p (b c) -> p b c", c=HD + 1)[:, :, 64:65]
            nc.vector.memset(vn_ones, 1.0)
            # warm the Exp activation table while the input DMAs stream
            tiny = cpool.tile([1, 1], f32, name="tiny")
            nc.vector.memset(tiny[:], 1.0)
            nc.scalar.activation(out=tiny[:], in_=tiny[:], func=FX.Exp)

            a2a_in = [dram.tile([NCORES, 128, CC_TOK[g]], bf16,
                                name=f"a2a_in{g}") for g in range(3)]
            a2a_out = [dram.tile([NCORES, 128, CC_TOK[g]], bf16,
                                 name=f"a2a_out{g}") for g in range(3)]

            # ---------------- filler machinery ----------------------------
            fillers = deque()   # (deadline_block_idx, closure)

            def pump(k):
                n = 0
                while fillers and n < k:
                    fillers.popleft()[1]()
                    n += 1

            def flush(deadline):
                while fillers and fillers[0][0] <= deadline:
                    fillers.popleft()[1]()

            # ---------------- phase 1: QKV projection ---------------------
            def load_x(t):
                xts = []
                for dk in range(DK):
                    xt = xpool.tile([128, QC], bf16, name=f"xt{t}_{dk}", tag="xt")
                    nc.sync.dma_start(
                        xt[:], xT_d.ap()[128 * dk:128 * (dk + 1),
                                         QC * t:QC * (t + 1)])
                    xts.append(xt)
                return xts

            def load_w(dk):
                nc.sync.dma_start(w_sb[:, 384 * dk:384 * (dk + 1)],
                                  wqkv_d.ap()[128 * dk:128 * (dk + 1), :])

            def queue_qkv(t, xts, deadline):
                """Queue matmul/bias/transpose closures for chunk t."""
                state = {}

                def mk_mm(m, dk):
                    def run():
                        if dk == 0:
                            state[m] = ps_q.tile([128, QC], f32,
                                                 name=f"qkv{t}_{m}", tag="psq")
                        c0 = 384 * dk + 128 * m
                        nc.tensor.matmul(state[m][:],
                                         w_sb[:, c0:c0 + 128],
                                         xts[dk][:],
                                         start=(dk == 0), stop=(dk == DK - 1))
                    return run

                def mk_bias(m):
                    def run():
                        ps = state[m]
                        bias_ap = bq_sb[:, m:m + 1]
                        if m == 0:
                            nc.vector.tensor_scalar_add(
                                out=qT[:, QC * t:QC * (t + 1)], in0=ps[:],
                                scalar1=bias_ap)
                        elif m == 1:
                            nc.vector.tensor_scalar_add(
                                out=kT[:, QC * t:QC * (t + 1)], in0=ps[:],
                                scalar1=bias_ap)
                        else:
                            vst = vpool.tile([128, QC], bf16,
                                             name=f"vst{t}", tag="vst")
                            state["v"] = vst
                            nc.vector.tensor_scalar_add(
                                out=vst[:], in0=ps[:], scalar1=bias_ap)
                    return run

                def mk_vtr(ci):
                    def run():
                        vst = state["v"]
                        gi = 4 * t + ci
                        trp = ps_q.tile([128, 128], bf16,
                                        name=f"tr{gi}", tag="psq")
                        nc.tensor.transpose(
                            trp[:], vst[:, 128 * ci:128 * (ci + 1)], ident[:])
                        dst = vn[:].rearrange(
                            "p (g c) -> p g c", c=HD + 1)[:, 2 * gi:2 * gi + 2, 0:64]
                        src = trp[:].rearrange("p (g c) -> p g c", c=64)
                        nc.vector.tensor_copy(out=dst, in_=src)
                    return run

                for m in range(3):
                    for dk in range(DK):
                        fillers.append((deadline, mk_mm(m, dk)))
                    fillers.append((deadline, mk_bias(m)))
                for ci in range(4):
                    fillers.append((deadline, mk_vtr(ci)))

            # ---------------- phase 2: attention block --------------------
            def att_store(att, bi):
                g = 0 if bi < 4 else (1 if bi < 6 else 2)
                tok = CC_TOK[g]
                base = 512 * (bi - CC_GROUPS[g][0])
                x0 = 0
                while x0 < 512:
                    flat = base + x0
                    e, off = divmod(flat, tok)
                    ln = min(512 - x0, tok - off)
                    nc.sync.dma_start(
                        a2a_in[g][e, :, off:off + ln], att[:, x0:x0 + ln])
                    x0 += ln

            cctr = [0]
            SLOT_PAIRS = [(0, 1), (0, 2), (1, 2)]

            def emit_block(b, j, bi):
                n_i = 4 * (j + 1)
                q0 = S * b + QC * j
                av01 = psav.tile([65, 2 * QC], f32, name=f"av{b}_{j}", tag="av")

                def emit_av(ea, eb, gi, i, off):
                    st, sp = (i == 0), (i == n_i - 1)
                    nc.tensor.matmul(av01[:, off:QC],
                                     vn[:, VW * gi:VW * gi + 65],
                                     e_all[:, QC * ea + off:QC * ea + QC],
                                     start=st, stop=sp,
                                     skip_group_check=True)
                    nc.tensor.matmul(av01[:, QC + off:2 * QC],
                                     vn[:, VW * gi + 65:VW * gi + 130],
                                     e_all[:, QC * eb + off:QC * eb + QC],
                                     start=st, stop=sp,
                                     skip_group_check=True)

                pend = []
                for i in range(n_i):
                    gi = (S // KC) * b + i
                    k0 = S * b + KC * i
                    m = i - 4 * j
                    off = 128 * m if m >= 0 else 0
                    sa, sb = SLOT_PAIRS[cctr[0] % 3]
                    ea = 2 * (cctr[0] % 3)
                    eb = ea + 1
                    cctr[0] += 1
                    nc.tensor.matmul(s_all[:, QC * sa + off:QC * sa + QC],
                                     kT[0:64, k0:k0 + KC],
                                     qT[0:64, q0 + off:q0 + QC],
                                     start=True, stop=True)
                    nc.tensor.matmul(s_all[:, QC * sb + off:QC * sb + QC],
                                     kT[64:128, k0:k0 + KC],
                                     qT[64:128, q0 + off:q0 + QC],
                                     start=True, stop=True)
                    stp = sb - sa
                    s3 = s_all[:].rearrange("p (c q) -> p c q", q=QC)[
                        :, sa:sb + 1:stp, off:QC]
                    e3 = e_all[:].rearrange("p (c q) -> p c q", q=QC)[
                        :, ea:eb + 1, off:QC]
                    nc.scalar.activation(out=e3, in_=s3, func=FX.Exp)
                    if m >= 0:
                        mk = masks_sb[:, 0:128]
                        c0, c1 = 128 * m, 128 * (m + 1)
                        nc.vector.tensor_tensor(
                            out=e_all[:, QC * ea + c0:QC * ea + c1],
                            in0=e_all[:, QC * ea + c0:QC * ea + c1], in1=mk,
                            op=ALU.mult)
                        nc.vector.tensor_tensor(
                            out=e_all[:, QC * eb + c0:QC * eb + c1],
                            in0=e_all[:, QC * eb + c0:QC * eb + c1], in1=mk,
                            op=ALU.mult)
                    if len(pend) >= (2 if bi < 7 else 1):
                        emit_av(*pend.pop(0))
                    pend.append((ea, eb, gi, i, off))
                    pump(4)
                while pend:
                    emit_av(*pend.pop(0))
                    pump(2)

                # softmax normalization: 1/d via column-form DVE reciprocal
                # (DMA row->col scatter + gather), partition-broadcast on
                # GpSimd, one multiply per head into bf16.
                d01 = rpool.tile([1, 2 * QC], f32, name=f"d{b}_{j}", tag="rr")
                nc.vector.tensor_copy(out=d01[:], in_=av01[64:65, :])
                dcol = rpool.tile([128, 8], f32, name=f"dc{b}_{j}", tag="dc")
                nc.sync.dma_start(dcol[:], d01[:])
                rcol = rpool.tile([128, 8], f32, name=f"rc{b}_{j}", tag="rc")
                nc.vector.reciprocal(out=rcol[:], in_=dcol[:])
                rr01 = rpool.tile([1, 2 * QC], f32, name=f"rr{b}_{j}", tag="rr2")
                nc.sync.dma_start(rr01[:], rcol[:])
                bs01 = bspool.tile([64, 2 * QC], f32, name=f"bs{b}_{j}", tag="bs")
                nc.gpsimd.partition_broadcast(bs01[:], rr01[:], channels=64)
                att = apool.tile([128, QC], bf16, name=f"att{b}_{j}", tag="att")
                nc.vector.tensor_tensor(out=att[0:64, :], in0=av01[0:64, 0:QC],
                                        in1=bs01[:, 0:QC], op=ALU.mult)
                nc.vector.tensor_tensor(out=att[64:128, :],
                                        in0=av01[0:64, QC:2 * QC],
                                        in1=bs01[:, QC:2 * QC], op=ALU.mult)
                att_store(att, bi)

            # ---------------- phase 3: out projection ---------------------
            def queue_outproj(ats_all, u_base, n_u, deadline, width,
                              u_in0=0):
                state = {}

                def mk_mm(u, dc, dk):
                    def run():
                        if dk == 0:
                            state[("ps", u, dc)] = ps_q.tile(
                                [128, QC], f32, name=f"op{u_base + u}_{dc}",
                                tag="psq")
                        ps = state[("ps", u, dc)]
                        nc.tensor.matmul(
                            ps[:], ats_all[:, width * dk + 128 * (u_in0 + u):
                                           width * dk + 128 * (u_in0 + u + 1)],
                            wo_sb[:, D * dk + QC * dc:D * dk + QC * dc + QC],
                            start=(dk == 0), stop=(dk == DK - 1))
                    return run

                def mk_out(u, dc):
                    def run():
                        ps = state[("ps", u, dc)]
                        osb = opool.tile([128, QC], f32,
                                         name=f"osb{u_base + u}_{dc}", tag="osb")
                        nc.vector.tensor_tensor(
                            out=osb[:], in0=ps[:],
                            in1=bo_bc[:, QC * dc:QC * (dc + 1)], op=ALU.add)
                        nc.sync.dma_start(
                            out_d.ap()[128 * (u_base + u):128 * (u_base + u + 1),
                                       QC * dc:QC * (dc + 1)], osb[:])
                    return run

                for u in range(n_u):
                    for dc in range(2):
                        for dk in range(DK):
                            fillers.append((deadline, mk_mm(u, dc, dk)))
                        fillers.append((deadline, mk_out(u, dc)))

            # ---------------- the schedule --------------------------------
            # prologue: interleave weight and x chunk-0 loads so the first
            # qkv matmul chain can start as soon as its operands land
            t0 = T_ORDER[0]
            xts_next = []
            for dk in range(DK):
                load_w(dk)
                xt = xpool.tile([128, QC], bf16, name=f"xt{t0}_{dk}", tag="xt")
                nc.sync.dma_start(
                    xt[:], xT_d.ap()[128 * dk:128 * (dk + 1),
                                     QC * t0:QC * (t0 + 1)])
                xts_next.append(xt)
                if dk == 1:
                    nc.sync.dma_start(bq_sb[:], bqkv_d.ap())
                    nc.sync.dma_start(ident[:], ident_d.ap())
                    nc.sync.dma_start(masks_sb[:], masks_d.ap())
            queue_qkv(t0, xts_next, deadline=0)
            flush(0)

            ats = [ppool.tile([128, DK * CC_TOK[g]], bf16, name=f"ats{g}")
                   for g in range(3)]

            def trigger_cc(g):
                nc.gpsimd.collective_compute(
                    "AllToAll", ALU.bypass,
                    replica_groups=[list(range(NCORES))],
                    ins=[a2a_in[g].opt()], outs=[a2a_out[g].opt()])

            def load_ats(g):
                def run():
                    nc.sync.dma_start(
                        ats[g][:], a2a_out[g][:].rearrange("e p c -> p e c"))
                return run

            for bi, (b, j) in enumerate(BLOCK_ORDER):
                flush(bi)
                if bi + 1 < len(BLOCK_ORDER):
                    xts_next = load_x(T_ORDER[bi + 1])
                    queue_qkv(T_ORDER[bi + 1], xts_next, deadline=bi + 1)
                if bi == 2:
                    def load_wo(dk):
                        def run():
                            nc.sync.dma_start(
                                wo_sb[:, D * dk:D * (dk + 1)],
                                wout_d.ap()[128 * dk:128 * (dk + 1), :])
                        return run
                    for dk in range(DK):
                        fillers.append((5, load_wo(dk)))

                    def load_bo():
                        nc.sync.dma_start(bo_sb[:], bout_d.ap())
                        nc.gpsimd.partition_broadcast(bo_bc[:], bo_sb[:],
                                                      channels=128)
                    fillers.append((5, lambda: load_bo()))
                if bi == 4:
                    trigger_cc(0)
                    fillers.append((99, load_ats(0)))
                if bi == 6:
                    trigger_cc(1)
                    fillers.append((99, load_ats(1)))
                if bi == 7:
                    queue_outproj(ats[0], 0, 1, deadline=99, width=CC_TOK[0])
                emit_block(b, j, bi)
            flush(99)
            trigger_cc(2)
            # deferred out-projection work bridges the tail collective
            queue_outproj(ats[0], 1, 1, deadline=100, width=CC_TOK[0],
                          u_in0=1)
            queue_outproj(ats[1], 2, 1, deadline=100, width=CC_TOK[1])
            flush(100)
            load_ats(2)()
            queue_outproj(ats[2], 3, 1, deadline=101, width=CC_TOK[2])
            flush(101)

    nc.finalize()
    return nc


def _host_inputs_causal(x, w_qkv, b_qkv, w_out, b_out):
    import ml_dtypes
    bf = ml_dtypes.bfloat16
    xT = np.ascontiguousarray(x.reshape(SL, D).T.astype(bf))
    const_ident = np.eye(128, dtype=bf)
    # keys on partitions (rows), queries on free dim (cols): valid iff
    # key <= query, i.e. the UPPER triangle inclusive.
    triu = np.triu(np.ones((128, 128), dtype=np.float32))
    const_masks = np.concatenate(
        [triu, np.zeros((128, 128), dtype=np.float32), triu], axis=1).astype(bf)
    wout_bf = np.ascontiguousarray(w_out.astype(bf))
    bo = np.ascontiguousarray(b_out.reshape(1, D).astype(np.float32))

    in_maps = []
    for c in range(NCORES):
        h0, h1 = 2 * c, 2 * c + 1

        def wcol(h, o):
            return w_qkv[:, 192 * h + o:192 * h + o + 64]

        def bcol(h, o):
            return b_qkv[192 * h + o:192 * h + o + 64]

        wq = np.concatenate([wcol(h0, 0), wcol(h1, 0)], axis=1) * np.float32(0.125)
        wk = np.concatenate([wcol(h0, 64), wcol(h1, 64)], axis=1)
        wv = np.concatenate([wcol(h0, 128), wcol(h1, 128)], axis=1)
        wc = np.ascontiguousarray(
            np.concatenate([wq, wk, wv], axis=1).astype(bf))
        bq = np.concatenate([bcol(h0, 0), bcol(h1, 0)]) * np.float32(0.125)
        bk = np.concatenate([bcol(h0, 64), bcol(h1, 64)])
        bv = np.concatenate([bcol(h0, 128), bcol(h1, 128)])
        bc = np.ascontiguousarray(np.stack([bq, bk, bv], axis=1),
                                  dtype=np.float32)

        in_maps.append({"xT": xT, "wqkv": wc, "bqkv": bc, "wout": wout_bf,
                        "bout": bo, "ident": const_ident,
                        "masks": const_masks})
    return in_maps


def _gather_causal(results):
    """Map per-core [512, 1024] outputs back to (B, S, D)."""
    out = np.empty((B, S, D), dtype=np.float32)
    for c in range(NCORES):
        r = results[c]["out"]
        for g, (s, e) in enumerate(CC_GROUPS):
            tok = CC_TOK[g]
            for row in range(tok):
                flat = tok * c + row
                k, xx = divmod(flat, 512)
                b, j = BLOCK_ORDER[s + k]
                out[b, 512 * j + xx, :] = r[CC_ROW0[g] + row]
    return out


# ======================================================================
# fallback (general/dense masks): original v1 builder
# ======================================================================

def _build_general(variant, exp_bias=0.0):
    assert variant in ("dense", "general")
    nc = bacc.Bacc("TRN2", target_bir_lowering=False, debug=False,
                   num_devices=NCORES)

    NK = S // KC
    NT = SL // QC
    EDT = f32r
    VDT = f32r

    xT_d = nc.dram_tensor("xT", [D, SL], f32, kind="ExternalInput")
    wqkv_d = nc.dram_tensor("wqkv", [D, 384], f32, kind="ExternalInput")
    bqkv_d = nc.dram_tensor("bqkv", [128, 3], f32, kind="ExternalInput")
    wout_d = nc.dram_tensor("wout", [D, D], f32, kind="ExternalInput")
    bout_d = nc.dram_tensor("bout", [1, D], f32, kind="ExternalInput")
    ident_d = nc.dram_tensor("ident", [128, 128], VDT, kind="ExternalInput")
    vones_d = nc.dram_tensor("vones", [128, 64], f32, kind="ExternalInput")
    maskT_d = nc.dram_tensor("maskT", [B, S, S], f32, kind="ExternalInput")
    out_d = nc.dram_tensor("out", [QC, D], f32, kind="ExternalOutput")

    with tile.TileContext(nc) as tc:
        with ExitStack() as stack:
            ep = stack.enter_context
            cpool = ep(tc.tile_pool(name="consts", bufs=1))
            big = ep(tc.tile_pool(name="big", bufs=1))
            xpool = ep(tc.tile_pool(name="xts", bufs=16))
            vpool = ep(tc.tile_pool(name="vstg", bufs=2))
            epool = ep(tc.tile_pool(name="epool", bufs=4))
            mpool = ep(tc.tile_pool(name="mpool", bufs=4))
            rpool = ep(tc.tile_pool(name="rpool", bufs=2))
            bcpool = ep(tc.tile_pool(name="bcpool", bufs=2))
            apool = ep(tc.tile_pool(name="apool", bufs=2))
            ppool = ep(tc.tile_pool(name="ppool", bufs=16))
            opool = ep(tc.tile_pool(name="opool", bufs=2))
            dram = ep(tc.tile_pool(name="dram", bufs=1, space="DRAM"))
            psmm = ep(tc.tile_pool(name="psmm", bufs=2, space="PSUM"))
            pssc = ep(tc.tile_pool(name="pssc", bufs=3, space="PSUM"))
            pstr = ep(tc.tile_pool(name="pstr", bufs=1, space="PSUM"))
            psav0 = ep(tc.tile_pool(name="psav0", bufs=1, space="PSUM"))
            psav1 = ep(tc.tile_pool(name="psav1", bufs=1, space="PSUM"))

            ident = cpool.tile([128, 128], VDT, name="ident")
            nc.sync.dma_start(ident[:], ident_d.ap())
            ones512 = cpool.tile([1, QC], f32r, name="ones512")
            nc.sync.dma_start(ones512[:], ones_d.ap().bitcast(f32r))
            bq_sb = cpool.tile([128, 3], f32, name="bq_sb")
            nc.sync.dma_start(bq_sb[:], bqkv_d.ap())
            w_sb = big.tile([128, DK * 384], f32r, name="w_sb")
            for dk in range(DK):
                nc.sync.dma_start(w_sb[:, 384 * dk:384 * (dk + 1)],
                                  wqkv_d.ap()[128 * dk:128 * (dk + 1), :]
                                  .bitcast(f32r))
            qT = big.tile([128, SL], f32r, name="qT")
            kT = big.tile([128, SL], f32r, name="kT")
            vn = big.tile([128, B * NK * VW], VDT, name="vn")
            vn_ones = vn[:].rearrange("p (b c) -> p b c", c=HD + 1)[:, :, 64:65]
            nc.sync.dma_start(vn_ones, vones_d.ap().bitcast(f32r))

            a2a_in = dram.tile([NCORES, 128, QC], f32, name="a2a_in")
            a2a_out = dram.tile([NCORES, 128, QC], f32, name="a2a_out")

            def emit_qkv(t):
                xts = []
                for dk in range(DK):
                    xt = xpool.tile([128, QC], f32r, name=f"xt{t}_{dk}", tag="xt")
                    nc.sync.dma_start(
                        xt[:], xT_d.ap()[128 * dk:128 * (dk + 1),
                                         QC * t:QC * (t + 1)].bitcast(f32r))
                    xts.append(xt)
                for m in range(3):
                    ps = psmm.tile([128, QC], f32, name=f"qkv{t}_{m}", tag="mm")
                    for dk in range(DK):
                        c0 = 384 * dk + 128 * m
                        nc.tensor.matmul(ps[:],
                                         w_sb[:, c0:c0 + 128],
                                         xts[dk][:],
                                         start=(dk == 0), stop=(dk == DK - 1))
                    bias_ap = bq_sb[:, m:m + 1]
                    if m == 0:
                        nc.vector.tensor_scalar_add(
                            out=qT[:, QC * t:QC * (t + 1)], in0=ps[:],
                            scalar1=bias_ap)
                    elif m == 1:
                        nc.vector.tensor_scalar_add(
                            out=kT[:, QC * t:QC * (t + 1)], in0=ps[:],
                            scalar1=bias_ap)
                    else:
                        vst = vpool.tile([128, QC], VDT, name=f"vst{t}", tag="vst")
                        nc.vector.tensor_scalar_add(out=vst[:], in0=ps[:],
                                                    scalar1=bias_ap)
                        for ci in range(4):
                            gi = 4 * t + ci
                            trp = pstr.tile([128, 128], VDT, name=f"tr{gi}",
                                            tag="tr")
                            nc.tensor.transpose(trp[:],
                                                vst[:, 128 * ci:128 * (ci + 1)],
                                                ident[:])
                            nc.vector.tensor_copy(
                                out=vn[:, VW * gi:VW * gi + 64],
                                in_=trp[:, 0:64])
                            nc.vector.tensor_copy(
                                out=vn[:, VW * gi + 65:VW * gi + 129],
                                in_=trp[:, 64:128])

            def emit_attn(b, j):
                n_i = S // KC
                q0 = S * b + QC * j
                av0 = psav0.tile([65, QC], f32, name=f"av0_{b}_{j}", tag="av0")
                av1 = psav1.tile([65, QC], f32, name=f"av1_{b}_{j}", tag="av1")

                def emit_av(e0, e1, gi, i):
                    st, sp_ = (i == 0), (i == n_i - 1)
                    nc.tensor.matmul(av0[:],
                                     vn[:, VW * gi:VW * gi + 65],
                                     e0[:], start=st, stop=sp_,
                                     skip_group_check=True)
                    nc.tensor.matmul(av1[:],
                                     vn[:, VW * gi + 65:VW * gi + 130],
                                     e1[:], start=st, stop=sp_,
                                     skip_group_check=True)

                pend = []
                for i in range(n_i):
                    gi = (S // KC) * b + i
                    k0 = S * b + KC * i
                    s0 = pssc.tile([128, QC], f32, name=f"s0_{b}_{j}_{i}", tag="sc")
                    s1 = pssc.tile([128, QC], f32, name=f"s1_{b}_{j}_{i}", tag="sc")
                    nc.tensor.matmul(s0[:], kT[0:64, k0:k0 + KC],
                                     qT[0:64, q0:q0 + QC],
                                     start=True, stop=True)
                    nc.tensor.matmul(s1[:], kT[64:128, k0:k0 + KC],
                                     qT[64:128, q0:q0 + QC],
                                     start=True, stop=True)
                    if variant == "general":
                        mt = mpool.tile([128, QC], f32, name=f"mt{b}_{j}_{i}",
                                        tag="mt")
                        nc.sync.dma_start(
                            mt[:], maskT_d.ap()[b, KC * i:KC * (i + 1),
                                                QC * j:QC * (j + 1)])
                        nc.vector.tensor_tensor(out=s0[:], in0=s0[:], in1=mt[:],
                                                op=ALU.add)
                        nc.vector.tensor_tensor(out=s1[:], in0=s1[:], in1=mt[:],
                                                op=ALU.add)
                    e0 = epool.tile([128, QC], EDT, name=f"e0_{b}_{j}_{i}", tag="e")
                    e1 = epool.tile([128, QC], EDT, name=f"e1_{b}_{j}_{i}", tag="e")
                    nc.scalar.activation(out=e0[:], in_=s0[:], func=FX.Exp,
                                         bias=exp_bias)
                    nc.scalar.activation(out=e1[:], in_=s1[:], func=FX.Exp,
                                         bias=exp_bias)
                    pend.append((e0, e1, gi, i))
                    if len(pend) > 1:
                        emit_av(*pend.pop(0))
                while pend:
                    emit_av(*pend.pop(0))

                def finalize():
                    l0 = rpool.tile([1, QC], f32, name=f"l0_{b}_{j}", tag="l0")
                    l1 = rpool.tile([1, QC], f32, name=f"l1_{b}_{j}", tag="l1")
                    nc.scalar.activation(out=l0[:], in_=av0[64:65, :], func=FX.Ln)
                    nc.scalar.activation(out=l1[:], in_=av1[64:65, :], func=FX.Ln)
                    rr0 = rpool.tile([1, QC], f32r, name=f"rr0_{b}_{j}", tag="rr0")
                    rr1 = rpool.tile([1, QC], f32r, name=f"rr1_{b}_{j}", tag="rr1")
                    nc.scalar.activation(out=rr0[:], in_=l0[:], func=FX.Exp,
                                         scale=-1.0)
                    nc.scalar.activation(out=rr1[:], in_=l1[:], func=FX.Exp,
                                         scale=-1.0)
                    bc0 = psmm.tile([128, QC], f32, name=f"bc0_{b}_{j}", tag="mm")
                    nc.tensor.matmul(bc0[:], ones512[0:1, 0:128], rr0[:],
                                     start=True, stop=True)
                    bc1 = psmm.tile([128, QC], f32, name=f"bc1_{b}_{j}", tag="mm")
                    nc.tensor.matmul(bc1[:], ones512[0:1, 0:128], rr1[:],
                                     start=True, stop=True)
                    bs = bcpool.tile([128, QC], f32, name=f"bs{b}_{j}", tag="bc")
                    nc.vector.tensor_copy(out=bs[0:64, :], in_=bc0[0:64, :])
                    nc.vector.tensor_copy(out=bs[64:128, :], in_=bc1[64:128, :])
                    att = apool.tile([128, QC], f32, name=f"att{b}_{j}", tag="att")
                    nc.vector.tensor_tensor(out=att[0:64, :], in0=av0[0:64, :],
                                            in1=bs[0:64, :], op=ALU.mult)
                    nc.vector.tensor_tensor(out=att[64:128, :], in0=av1[0:64, :],
                                            in1=bs[64:128, :], op=ALU.mult)
                    nc.sync.dma_start(a2a_in[NQ * b + j], att[:])

                return finalize

            blocks = [(b, j) for b in range(B) for j in range(NQ)]
            for t in range(NT):
                emit_qkv(t)
                if t >= 1:
                    emit_attn(*blocks[t - 1])()
            emit_attn(*blocks[NT - 1])()

            wo_sb = big.tile([128, DK * D], f32r, name="wo_sb")
            for dk in range(DK):
                nc.sync.dma_start(wo_sb[:, D * dk:D * (dk + 1)],
                                  wout_d.ap()[128 * dk:128 * (dk + 1), :]
                                  .bitcast(f32r))
            bo_sb = cpool.tile([1, D], f32r, name="bo_sb")
            nc.sync.dma_start(bo_sb[:], bout_d.ap().bitcast(f32r))
            nc.gpsimd.collective_compute(
                "AllToAll", ALU.bypass,
                replica_groups=[list(range(NCORES))],
                ins=[a2a_in.opt()], outs=[a2a_out.opt()])

            for qsub in range(4):
                ats = []
                for dk in range(DK):
                    at = ppool.tile([128, 128], f32r, name=f"at{qsub}_{dk}",
                                    tag="at")
                    nc.sync.dma_start(
                        at[:], a2a_out[dk, :, 128 * qsub:128 * (qsub + 1)]
                        .bitcast(f32r))
                    ats.append(at)
                for dc in range(2):
                    ps = psmm.tile([128, QC], f32, name=f"op{qsub}_{dc}", tag="mm")
                    for dk in range(DK):
                        c0 = D * dk + QC * dc
                        nc.tensor.matmul(ps[:], ats[dk][:],
                                         wo_sb[:, c0:c0 + QC],
                                         start=(dk == 0), stop=False)
                    nc.tensor.matmul(ps[:], ones512[0:1, 0:128],
                                     bo_sb[0:1, QC * dc:QC * (dc + 1)],
                                     start=False, stop=True)
                    osb = opool.tile([128, QC], f32, name=f"osb{qsub}_{dc}",
                                     tag="osb")
                    nc.vector.tensor_copy(out=osb[:], in_=ps[:])
                    nc.sync.dma_start(
                        out_d.ap()[128 * qsub:128 * (qsub + 1),
                                   QC * dc:QC * (dc + 1)], osb[:])

    nc.finalize()
    return nc


def _detect_variant(mask):
    if not mask.any():
        return "dense"
    tri = np.where(np.tril(np.ones((S, S), dtype=bool)),
                   np.float32(0.0), np.float32(-1e9)).astype(np.float32)
    for b in range(B):
        if not np.array_equal(mask[b], tri):
            return "general"
    return "causal"


def _run_general(variant, x, mask, w_qkv, b_qkv, w_out, b_out):
    global LAST_EXEC_NS, LAST_RESULTS
    # guard exp() against overflow: bound max score via norms; the shift is
    # folded into the (transposed) additive mask.
    xf = x.reshape(SL, D)
    qkv = xf @ w_qkv + b_qkv
    qkv = qkv.reshape(SL, H, 3 * HD)
    qn = np.linalg.norm(qkv[:, :, :HD], axis=2).max()
    kn = np.linalg.norm(qkv[:, :, HD:2 * HD], axis=2).max()
    mmax = 0.0 if variant == "dense" else max(0.0, float(np.nanmax(mask)))
    bound = qn * kn / np.sqrt(HD) + mmax
    shift = min(0.0, 60.0 - bound)
    if variant == "dense":
        maskT = np.broadcast_to(np.float32(shift), (B, S, S)).copy()
    else:
        maskT = np.ascontiguousarray(
            mask.transpose(0, 2, 1) + np.float32(shift))

    xT = np.ascontiguousarray(x.reshape(SL, D).T)
    const_ident = np.eye(128, dtype=np.float32)
    const_ones = np.ones((1, QC), dtype=np.float32)
    const_vones = np.ones((128, 64), dtype=np.float32)
    bo = np.ascontiguousarray(b_out.reshape(1, D))

    in_maps = []
    for c in range(NCORES):
        h0, h1 = 2 * c, 2 * c + 1

        def wcol(h, o):
            return w_qkv[:, 192 * h + o:192 * h + o + 64]

        def bcol(h, o):
            return b_qkv[192 * h + o:192 * h + o + 64]

        wq = np.concatenate([wcol(h0, 0), wcol(h1, 0)], axis=1) * np.float32(0.125)
        wk = np.concatenate([wcol(h0, 64), wcol(h1, 64)], axis=1)
        wv = np.concatenate([wcol(h0, 128), wcol(h1, 128)], axis=1)
        wc = np.ascontiguousarray(np.concatenate([wq, wk, wv], axis=1))
        bq = np.concatenate([bcol(h0, 0), bcol(h1, 0)]) * np.float32(0.125)
        bk = np.concatenate([bcol(h0, 64), bcol(h1, 64)])
        bv = np.concatenate([bcol(h0, 128), bcol(h1, 128)])
        bc = np.ascontiguousarray(np.stack([bq, bk, bv], axis=1))

        m = {"xT": xT, "wqkv": wc, "bqkv": bc, "wout": w_out, "bout": bo,
             "ident": const_ident, "ones": const_ones, "vones": const_vones,
             "maskT": maskT}
        in_maps.append(m)

    nc = _build_general("general")
    trace = os.environ.get("SMSA_TRACE", "0") == "1"
    res = bass_utils.run_bass_kernel_spmd(
        nc, in_maps, core_ids=list(range(NCORES)), trace=trace)
    LAST_EXEC_NS = res.exec_time_ns
    LAST_RESULTS = res
    parts = [res.results[c]["out"] for c in range(NCORES)]
    out = np.concatenate(parts, axis=0).reshape(B, S, D)
    return np.ascontiguousarray(out.astype(np.float32, copy=False))


def kernel(**inputs):
    global LAST_EXEC_NS, LAST_RESULTS
    x = np.ascontiguousarray(np.asarray(inputs["x"], dtype=np.float32))
    mask = np.asarray(inputs["mask"], dtype=np.float32)
    w_qkv = np.asarray(inputs["w_qkv"], dtype=np.float32)
    b_qkv = np.asarray(inputs["b_qkv"], dtype=np.float32)
    w_out = np.ascontiguousarray(np.asarray(inputs["w_out"], dtype=np.float32))
    b_out = np.asarray(inputs["b_out"], dtype=np.float32)

    variant = _detect_variant(mask)
    if variant != "causal":
        return _run_general(variant, x, mask, w_qkv, b_qkv, w_out, b_out)

    in_maps = _host_inputs_causal(x, w_qkv, b_qkv, w_out, b_out)
    nc = _build_causal()
    trace = os.environ.get("SMSA_TRACE", "0") == "1"
    tc_env = os.environ.get("SMSA_TRACE_CORES", "")
    kw = {}
    if tc_env:
        kw["trace_cores"] = [int(c) for c in tc_env.split(",")]
    res = bass_utils.run_bass_kernel_spmd(
        nc, in_maps, core_ids=list(range(NCORES)), trace=trace, **kw)
    LAST_EXEC_NS = res.exec_time_ns
    LAST_RESULTS = res
    out = _gather_causal(res.results)
    return np.ascontiguousarray(out)
